# revision 1
# baseline (speedup 1.0000x reference)
"""GIN conv + 2 GCN heads (VGAE-style encoder) on 8 Trainium2 NeuronCores.

Strategy (memory-regime, gather-bound):
  - Nodes are permuted (degree-balanced round-robin) and sharded into
    8 cores x 98 blocks x 128 slots = 100352 positions.
  - Edges are assigned to the core owning their destination; per core they
    are split into 4 source-quadrant streams (int16 gather index limit) and
    sorted by destination block.
  - Launch 1 (GIN + MLP): per 128-edge chunk, dma_gather x[src] rows
    (512B each), build a one-hot [edges, dst_slot] matrix on the vector
    engine (iota == dst compare), and matmul-accumulate into a PSUM tile
    [feat, 128 nodes].  Self-edges fold the "+x_i" term into the same path.
    The per-block PSUM then flows through the MLP (W1/BN/relu/W2/relu) and
    the two GCN head weight matmuls, producing y = [h@Wmu | h@Wls] rows.
  - Host gathers y from all cores (the halo exchange).
  - Launch 2 (GCN aggregation): same machinery gathering y rows, with the
    one-hot scaled by the GCN norm coefficients (self-loops included as
    edges), node-major PSUM accumulation, plus bias.
"""

import sys
import time
import hashlib
from contextlib import ExitStack

sys.path.insert(0, "/opt/trn_rl_repo")

import numpy as np
from concourse import bacc, mybir
import concourse.tile as tile
from concourse.bass_utils import run_bass_kernel_spmd
from concourse.masks import make_identity

P = 128
NCORES = 8
N = 100000
DIN = 128
DH = 128
DOUT = 64
NPB = 98                  # node blocks per core
NPC = NPB * P             # 12544 nodes per core
NPAD = NCORES * NPC       # 100352 padded node positions
NQ = 4                    # source quadrants (int16 index range)
QS = NPAD // NQ           # 25088 rows per quadrant (< 32768)
CALL = 2048               # gather indices per dma_gather call
CPC = CALL // P           # chunks per call (32)
F32 = mybir.dt.float32
BF16 = mybir.dt.bfloat16
NP_BF16 = mybir.dt.np(mybir.dt.bfloat16)
I16 = mybir.dt.int16
I32 = mybir.dt.int32


# ----------------------------------------------------------------------------
# host-side preprocessing
# ----------------------------------------------------------------------------

def _permute_nodes(dst):
    """Degree-balanced node permutation: sort by in-degree, deal round-robin
    over the 784 (core, block) windows.  Returns pos[n] in [0, NPAD)."""
    deg = np.bincount(dst, minlength=N)
    order = np.argsort(-deg, kind="stable")
    rank = np.empty(N, np.int64)
    rank[order] = np.arange(N)
    nwin = NCORES * NPB
    win = rank % nwin
    slot = rank // nwin
    core = win % NCORES
    block = win // NCORES
    pos = core * NPC + block * P + slot
    return pos, deg


def _pack_stream(srcidx, dstslot, norm, counts_by_block, cpb):
    """Lay out one (core, quadrant) stream: edges already sorted by dst
    block; pad each block group to cpb[b]*128 positions, pad the stream to a
    CALL multiple.  Returns (idx16 [ncalls*128, CALL//16], dst32
    [ncalls*128, CPC], nrm32 or None)."""
    total_chunks = int(cpb.sum())
    ncalls = max(1, -(-total_chunks // CPC))
    tot = ncalls * CALL
    sidx = np.zeros(tot, np.int16)
    sdst = np.full(tot, -1.0, np.float32)
    snrm = np.zeros(tot, np.float32) if norm is not None else None
    # scatter block groups into their padded spans
    out_off = np.concatenate([[0], np.cumsum(cpb[:-1] * P)])
    in_off = np.concatenate([[0], np.cumsum(counts_by_block[:-1])])
    for b in range(NPB):
        c = int(counts_by_block[b])
        if c == 0:
            continue
        o, i = int(out_off[b]), int(in_off[b])
        sidx[o:o + c] = srcidx[i:i + c]
        sdst[o:o + c] = dstslot[i:i + c]
        if snrm is not None:
            snrm[o:o + c] = norm[i:i + c]
    # pack per call
    idx16 = np.concatenate([
        np.tile(sidx[k * CALL:(k + 1) * CALL].reshape(CALL // 16, 16).T, (8, 1))
        for k in range(ncalls)
    ], axis=0)
    dst32 = np.concatenate([
        sdst[k * CALL:(k + 1) * CALL].reshape(CPC, P).T.copy()
        for k in range(ncalls)
    ], axis=0)
    nrm32 = None
    if snrm is not None:
        nrm32 = np.concatenate([
            snrm[k * CALL:(k + 1) * CALL].reshape(CPC, P).T.copy()
            for k in range(ncalls)
        ], axis=0)
    return idx16, dst32, nrm32, ncalls


def _build_streams(src_gidx, dstblock, dstslot, norm, ecore):
    """Split per (core, quadrant), sort by dst block, compute shared chunk
    structure, pack arrays.

    src_gidx: gather index WITHIN quadrant (int), equantum: quadrant id per
    edge is src_gidx // QS handled by caller: here src_gidx is (qid, idx).
    """
    qid, sidx = src_gidx
    counts = np.zeros((NCORES, NQ, NPB), np.int64)
    per = {}
    for k in range(NCORES):
        mk = ecore == k
        for q in range(NQ):
            m = mk & (qid == q)
            sb = dstblock[m]
            o = np.lexsort((sidx[m], sb))
            per[(k, q)] = (
                sidx[m][o].astype(np.int16),
                dstslot[m][o].astype(np.float32),
                None if norm is None else norm[m][o].astype(np.float32),
            )
            counts[k, q] = np.bincount(sb, minlength=NPB)
    # shared chunk structure: per (q, b) max over cores
    cpb = -(-counts.max(axis=0) // P)          # [NQ, NPB] chunks per block
    cpb[0] = np.maximum(cpb[0], 1)             # q=0 initializes each PSUM tile
    packed = {}
    ncalls = np.zeros(NQ, np.int64)
    for q in range(NQ):
        for k in range(NCORES):
            si, sd, nr = per[(k, q)]
            idx16, dst32, nrm32, nc_ = _pack_stream(si, sd, nr, counts[k, q], cpb[q])
            packed[(k, q)] = (idx16, dst32, nrm32)
            ncalls[q] = nc_
    # program structure: chunks per block, with stream tail padding assigned
    # to the last block
    cprog = cpb.copy()
    for q in range(NQ):
        cprog[q, NPB - 1] += ncalls[q] * CPC - int(cpb[q].sum())
    return packed, cprog, ncalls


# ----------------------------------------------------------------------------
# device programs
# ----------------------------------------------------------------------------

def _emit_aggregation(nc, tc, ctx, x_in, idx_ins, dst_ins, nrm_ins, cprog,
                      ncalls, per_block_fn, name):
    """Shared skeleton: stream gathers + one-hot matmul accumulation.

    per_block_fn(b, psum, pools) consumes the finished PSUM tile of block b.
    If nrm_ins is not None the one-hot is scaled by the norm stream and the
    matmul orientation is node-major (lhsT=onehot); otherwise feature-major
    (lhsT=payload).
    """
    node_major = nrm_ins is not None

    const = ctx.enter_context(tc.tile_pool(name=f"{name}_const", bufs=1))
    iota_i = const.tile([P, P], I32, tag="iota_i")
    nc.gpsimd.iota(iota_i[:], pattern=[[1, P]], base=0, channel_multiplier=0)
    iota_f = const.tile([P, P], BF16, tag="iota_f")
    nc.vector.tensor_copy(iota_f[:], iota_i[:])

    pay_pools = [
        ctx.enter_context(tc.tile_pool(name=f"{name}_pay{q}", bufs=2))
        for q in range(NQ)
    ]
    meta_pools = [
        ctx.enter_context(tc.tile_pool(name=f"{name}_meta{q}", bufs=2))
        for q in range(NQ)
    ]
    oh_pool = ctx.enter_context(tc.tile_pool(name=f"{name}_oh", bufs=4))
    psum_pool = ctx.enter_context(
        tc.tile_pool(name=f"{name}_psum", bufs=2, space="PSUM"))
    aux = {}

    class Stream:
        def __init__(self, q):
            self.q = q
            self.next_chunk = 0
            self.cur_call = -1
            self.pay = self.dst = self.nrm = None

        def ensure(self):
            call = self.next_chunk // CPC
            if call != self.cur_call:
                self.cur_call = call
                q = self.q
                idx_t = meta_pools[q].tile([P, CALL // 16], I16, tag="idx")
                nc.sync.dma_start(
                    out=idx_t[:], in_=idx_ins[q][call * P:(call + 1) * P, :])
                self.dst = meta_pools[q].tile([P, CPC], F32, tag="dst")
                nc.sync.dma_start(
                    out=self.dst[:], in_=dst_ins[q][call * P:(call + 1) * P, :])
                if node_major:
                    self.nrm = meta_pools[q].tile([P, CPC], F32, tag="nrm")
                    nc.sync.dma_start(
                        out=self.nrm[:],
                        in_=nrm_ins[q][call * P:(call + 1) * P, :])
                self.pay = pay_pools[q].tile([P, CPC, DIN], BF16, tag="pay")
                nc.gpsimd.dma_gather(
                    self.pay[:], x_in[q * QS:(q + 1) * QS, :], idx_t[:],
                    CALL, CALL, DIN, single_packet=False, queue_num=q)

        def consume(self):
            self.ensure()
            t = self.next_chunk
            self.next_chunk += 1
            return self.pay, self.dst, self.nrm, t % CPC

    streams = [Stream(q) for q in range(NQ)]

    for b in range(NPB):
        psum = psum_pool.tile([P, P], F32, tag="agg")
        cells = [(q, int(cprog[q][b])) for q in range(NQ) if cprog[q][b] > 0]
        nchunks = sum(c for _, c in cells)
        done = 0
        for q, cnt in cells:
            st = streams[q]
            for _ in range(cnt):
                pay, dstt, nrmt, cl = st.consume()
                oh = oh_pool.tile([P, P], BF16, tag="oh")
                if node_major:
                    nc.vector.tensor_scalar(
                        out=oh[:], in0=iota_f[:],
                        scalar1=dstt[:, cl:cl + 1], scalar2=nrmt[:, cl:cl + 1],
                        op0=mybir.AluOpType.is_equal, op1=mybir.AluOpType.mult)
                    nc.tensor.matmul(
                        psum[:], lhsT=oh[:], rhs=pay[:, cl, :],
                        start=(done == 0), stop=(done == nchunks - 1))
                else:
                    nc.vector.tensor_scalar(
                        out=oh[:], in0=iota_f[:],
                        scalar1=dstt[:, cl:cl + 1], scalar2=None,
                        op0=mybir.AluOpType.is_equal)
                    nc.tensor.matmul(
                        psum[:], lhsT=pay[:, cl, :], rhs=oh[:],
                        start=(done == 0), stop=(done == nchunks - 1))
                done += 1
        per_block_fn(b, psum, aux)


def build_launch1(cprog, ncalls):
    """GIN aggregation + MLP + head matmuls -> y rows (node-major)."""
    nc = bacc.Bacc(dynamic_dma_scratch_size=65536, num_swdge_queues=4)
    x_in = nc.declare_dram_parameter("x", [NPAD, DIN], BF16, isOutput=False)
    idx_ins, dst_ins = [], []
    for q in range(NQ):
        idx_ins.append(nc.declare_dram_parameter(
            f"idx{q}", [int(ncalls[q]) * P, CALL // 16], I16, isOutput=False))
        dst_ins.append(nc.declare_dram_parameter(
            f"dst{q}", [int(ncalls[q]) * P, CPC], F32, isOutput=False))
    w1_in = nc.declare_dram_parameter("w1", [DIN, DH], F32, isOutput=False)
    w2_in = nc.declare_dram_parameter("w2", [DH, DH], F32, isOutput=False)
    w3_in = nc.declare_dram_parameter("w3", [DH, 2 * DOUT], F32, isOutput=False)
    vec_in = nc.declare_dram_parameter("vecs", [DH, 3], F32, isOutput=False)
    y_out = nc.declare_dram_parameter("y", [NPC, 2 * DOUT], BF16, isOutput=True)

    with ExitStack() as ctx:
        tc = ctx.enter_context(tile.TileContext(nc))
        wp = ctx.enter_context(tc.tile_pool(name="weights", bufs=1))
        w1 = wp.tile([DIN, DH], F32, tag="w1")
        nc.sync.dma_start(out=w1[:], in_=w1_in[:])
        w2 = wp.tile([DH, DH], F32, tag="w2")
        nc.sync.dma_start(out=w2[:], in_=w2_in[:])
        w3 = wp.tile([DH, 2 * DOUT], F32, tag="w3")
        nc.sync.dma_start(out=w3[:], in_=w3_in[:])
        # per-partition scalar columns [DH, 1]: BN scale, BN shift, b2
        vcols = wp.tile([DH, 3], F32, tag="vcols")
        nc.sync.dma_start(out=vcols[:], in_=vec_in[:])
        ident = wp.tile([P, P], F32, tag="ident")
        make_identity(nc, ident[:])
        s_col = vcols[:, 0:1]
        t_col = vcols[:, 1:2]
        b2_col = vcols[:, 2:3]

        mlp = ctx.enter_context(tc.tile_pool(name="mlp", bufs=2))
        mpsum = ctx.enter_context(
            tc.tile_pool(name="mpsum", bufs=2, space="PSUM"))

        def per_block(b, psum, aux):
            h0 = mlp.tile([DIN, P], F32, tag="h0")
            nc.scalar.activation(h0[:], psum[:],
                                 mybir.ActivationFunctionType.Copy)
            p2 = mpsum.tile([DH, P], F32, tag="mp")
            nc.tensor.matmul(p2[:], lhsT=w1[:], rhs=h0[:], start=True, stop=True)
            h1 = mlp.tile([DH, P], F32, tag="h1")
            nc.scalar.activation(h1[:], p2[:],
                                 mybir.ActivationFunctionType.Relu,
                                 bias=t_col, scale=s_col)
            p3 = mpsum.tile([DH, P], F32, tag="mp")
            nc.tensor.matmul(p3[:], lhsT=w2[:], rhs=h1[:], start=True, stop=True)
            h2 = mlp.tile([DH, P], F32, tag="h2")
            nc.scalar.activation(h2[:], p3[:],
                                 mybir.ActivationFunctionType.Relu,
                                 bias=b2_col, scale=1.0)
            p4 = mpsum.tile([2 * DOUT, P], F32, tag="mp")
            nc.tensor.matmul(p4[:], lhsT=w3[:], rhs=h2[:], start=True, stop=True)
            yt = mlp.tile([2 * DOUT, P], F32, tag="yt")
            nc.scalar.activation(yt[:], p4[:],
                                 mybir.ActivationFunctionType.Copy)
            p5 = mpsum.tile([P, 2 * DOUT], F32, tag="p5")
            nc.tensor.transpose(p5[:], yt[:], ident[:])
            yn = mlp.tile([P, 2 * DOUT], BF16, tag="yn")
            nc.scalar.activation(yn[:], p5[:],
                                 mybir.ActivationFunctionType.Copy)
            nc.sync.dma_start(out=y_out[b * P:(b + 1) * P, :], in_=yn[:])

        _emit_aggregation(nc, tc, ctx, x_in, idx_ins, dst_ins, None, cprog,
                          ncalls, per_block, "l1")
    nc.finalize()
    return nc


def build_launch2(cprog, ncalls):
    """GCN aggregation of y rows with norm scaling + bias (node-major)."""
    nc = bacc.Bacc(dynamic_dma_scratch_size=65536, num_swdge_queues=4)
    y_in = nc.declare_dram_parameter("y", [NPAD, 2 * DOUT], BF16, isOutput=False)
    idx_ins, dst_ins, nrm_ins = [], [], []
    for q in range(NQ):
        idx_ins.append(nc.declare_dram_parameter(
            f"idx{q}", [int(ncalls[q]) * P, CALL // 16], I16, isOutput=False))
        dst_ins.append(nc.declare_dram_parameter(
            f"dst{q}", [int(ncalls[q]) * P, CPC], F32, isOutput=False))
        nrm_ins.append(nc.declare_dram_parameter(
            f"nrm{q}", [int(ncalls[q]) * P, CPC], F32, isOutput=False))
    bias_in = nc.declare_dram_parameter("bias", [1, 2 * DOUT], F32,
                                        isOutput=False)
    out = nc.declare_dram_parameter("out", [NPC, 2 * DOUT], F32, isOutput=True)

    with ExitStack() as ctx:
        tc = ctx.enter_context(tile.TileContext(nc))
        wp = ctx.enter_context(tc.tile_pool(name="biasp", bufs=1))
        # broadcast bias row across partitions: ones[1,P].T @ bias[1,128]
        bias_row = wp.tile([1, 2 * DOUT], F32, tag="bias_row")
        nc.sync.dma_start(out=bias_row[:], in_=bias_in[:])
        ones_row = wp.tile([1, P], F32, tag="ones_row")
        nc.gpsimd.memset(ones_row[:], 1.0)
        bpsum_pool = ctx.enter_context(
            tc.tile_pool(name="bpsum", bufs=1, space="PSUM"))
        bias_ps = bpsum_pool.tile([P, 2 * DOUT], F32, tag="bps")
        nc.tensor.matmul(bias_ps[:], lhsT=ones_row[:], rhs=bias_row[:],
                         start=True, stop=True)
        bias_t = wp.tile([P, 2 * DOUT], F32, tag="bias")
        nc.scalar.activation(bias_t[:], bias_ps[:],
                             mybir.ActivationFunctionType.Copy)
        fin = ctx.enter_context(tc.tile_pool(name="fin", bufs=2))

        def per_block(b, psum, aux):
            ob = fin.tile([P, 2 * DOUT], F32, tag="ob")
            nc.vector.tensor_tensor(out=ob[:], in0=psum[:], in1=bias_t[:],
                                    op=mybir.AluOpType.add)
            nc.sync.dma_start(out=out[b * P:(b + 1) * P, :], in_=ob[:])

        _emit_aggregation(nc, tc, ctx, y_in, idx_ins, dst_ins, nrm_ins, cprog,
                          ncalls, per_block, "l2")
    nc.finalize()
    return nc


# ----------------------------------------------------------------------------
# entry point
# ----------------------------------------------------------------------------

_CACHE = {}
LAST_TIMES = {}


def make_in_maps1(prep):
    packed1, _, _ = prep["l1"]
    in_maps1 = []
    for k in range(NCORES):
        m = {"x": prep["x_pad"], "w1": prep["W1"], "w2": prep["W2"],
             "w3": prep["w3"], "vecs": prep["vecs"]}
        for q in range(NQ):
            idx16, dst32, _ = packed1[(k, q)]
            m[f"idx{q}"] = idx16
            m[f"dst{q}"] = dst32
        in_maps1.append(m)
    return in_maps1


def make_in_maps2(prep, y_full):
    packed2, _, _ = prep["l2"]
    in_maps2 = []
    for k in range(NCORES):
        m = {"y": y_full, "bias": prep["bias"]}
        for q in range(NQ):
            idx16, dst32, nrm32 = packed2[(k, q)]
            m[f"idx{q}"] = idx16
            m[f"dst{q}"] = dst32
            m[f"nrm{q}"] = nrm32
        in_maps2.append(m)
    return in_maps2


def _prepare(x, edge_index, W1, b1, gamma, beta, rmean, rvar, W2, b2,
             Wmu, bmu, Wls, bls):
    src = np.ascontiguousarray(edge_index[0]).astype(np.int64)
    dst = np.ascontiguousarray(edge_index[1]).astype(np.int64)
    pos, deg_in = _permute_nodes(dst)
    core_of = pos // NPC
    block_of = (pos % NPC) // P
    slot_of = pos % P

    nodes = np.arange(N, dtype=np.int64)
    # ---- launch 1 edge streams: edges + self edges, gather x by ORIGINAL id
    s1 = np.concatenate([src, nodes])
    d1 = np.concatenate([dst, nodes])
    l1 = _build_streams(
        (s1 // QS, s1 % QS), block_of[d1], slot_of[d1], None, core_of[d1])

    # ---- launch 2: same edges + self loops, gather y by PERMUTED position
    deg = deg_in.astype(np.float64) + 1.0
    dinv = 1.0 / np.sqrt(deg)
    nrm_e = (dinv[src] * dinv[dst]).astype(np.float32)
    nrm_s = (dinv * dinv)[nodes].astype(np.float32)
    sp = np.concatenate([pos[src], pos[nodes]])
    d2 = np.concatenate([dst, nodes])
    nrm = np.concatenate([nrm_e, nrm_s])
    l2 = _build_streams(
        (sp // QS, sp % QS), block_of[d2], slot_of[d2], nrm, core_of[d2])

    # ---- dense host data
    x_pad = np.zeros((NPAD, DIN), NP_BF16)
    x_pad[:N] = x.astype(NP_BF16)
    eps = 1e-5
    s64 = gamma.astype(np.float64) / np.sqrt(rvar.astype(np.float64) + eps)
    # BN(z + b1) = s*z + (s*(b1 - rmean) + beta)
    t64 = s64 * (b1.astype(np.float64) - rmean.astype(np.float64)) \
        + beta.astype(np.float64)
    s = s64.astype(np.float32)
    t = t64.astype(np.float32)
    w3 = np.concatenate([Wmu, Wls], axis=1).astype(np.float32)
    vecs = np.ascontiguousarray(
        np.stack([s, t, b2.astype(np.float32)], axis=1))  # [DH, 3]
    bias = np.concatenate([bmu, bls]).astype(np.float32)[None, :]
    return dict(pos=pos, l1=l1, l2=l2, x_pad=x_pad,
                W1=np.ascontiguousarray(W1, np.float32),
                W2=np.ascontiguousarray(W2, np.float32),
                w3=w3, vecs=vecs, bias=bias)


def kernel(**inputs):
    key = hashlib.sha1(
        np.ascontiguousarray(inputs["edge_index"]).tobytes()).hexdigest()
    if key not in _CACHE:
        prep = _prepare(**inputs)
        packed1, cprog1, ncalls1 = prep["l1"]
        packed2, cprog2, ncalls2 = prep["l2"]
        nc1 = build_launch1(cprog1, ncalls1)
        nc2 = build_launch2(cprog2, ncalls2)
        _CACHE[key] = (prep, nc1, nc2)
    prep, nc1, nc2 = _CACHE[key]
    packed1, cprog1, ncalls1 = prep["l1"]
    packed2, cprog2, ncalls2 = prep["l2"]

    in_maps1 = make_in_maps1(prep)
    t0 = time.time()
    res1 = run_bass_kernel_spmd(nc1, in_maps1, list(range(NCORES)))
    LAST_TIMES["launch1_wall_s"] = time.time() - t0
    y_full = np.concatenate([res1.results[k]["y"] for k in range(NCORES)],
                            axis=0)

    in_maps2 = make_in_maps2(prep, y_full)
    t0 = time.time()
    res2 = run_bass_kernel_spmd(nc2, in_maps2, list(range(NCORES)))
    LAST_TIMES["launch2_wall_s"] = time.time() - t0
    out_full = np.concatenate([res2.results[k]["out"] for k in range(NCORES)],
                              axis=0)

    final = out_full[prep["pos"][:N]]
    return np.ascontiguousarray(final[:, :DOUT]), \
        np.ascontiguousarray(final[:, DOUT:])



# revision 5
# speedup vs baseline: 1.6981x; 1.6981x over previous
"""GIN conv + 2 GCN heads (VGAE-style encoder) on 8 Trainium2 NeuronCores.

Strategy (memory-regime, gather-bound; v2):
  - Nodes are permuted (degree-balanced round-robin) into 8 cores x 98
    blocks x 128 slots; x is stored in permuted order so per-block rows
    are dense.
  - Edges are assigned to the core owning their destination and split
    into 2 source-range streams (signed-int16 gather index, base offset
    +-32768), sorted by destination block.
  - The per-chunk one-hot matrices (128 edges -> 128 dst slots) are
    precomputed HOST-side as an fp8 stream and DMA'd in dense 512KB
    groups: no per-chunk vector-engine work on device.
  - Launch 1 (GIN + MLP): per 128-edge chunk, dma_gather x rows (256B)
    and matmul-accumulate payT @ onehot into a feature-major PSUM tile.
    The self term "+x_i" is a dense identity matmul of the block's x
    rows.  The PSUM flows through the (bf16) MLP + fused BN and both
    GCN head weights, then rows are scaled by dinv -> yhat.
  - Host gathers yhat from all cores (the halo exchange).
  - Launch 2 (GCN): same streams/one-hots gather yhat rows, node-major
    matmul accumulation; out = dinv*psum + (dinv*yhat_blk + bias).
    (norm(s,d) = dinv_s*dinv_d is folded into yhat and the dinv post-
    scale; self-loops are the dense dinv*yhat term.)
"""

import sys
import time
import hashlib
from contextlib import ExitStack

sys.path.insert(0, "/opt/trn_rl_repo")

import numpy as np
from concourse import bacc, mybir
import concourse.tile as tile
from concourse.bass_utils import run_bass_kernel_spmd

P = 128
NCORES = 8
N = 100000
DIN = 128
DH = 128
DOUT = 64
NPB = 98                  # node blocks per core
NPC = NPB * P             # 12544 nodes per core
NPAD = NCORES * NPC       # 100352 padded node positions
NQ = 4                    # source streams (unsigned int16 gather index)
# segment sizes tuned so per-(stream, block) edge counts sit mid-chunk
# (means ~4.5, 4.5, 4.5, 2.5 chunks) to minimise ceil padding
QSEG = [28224, 28224, 28224, 15680]
QBASE = [0, 28224, 56448, 84672]
QBOUND = [28224, 56448, 84672, 100352]
CALL = 4096               # gather indices per dma_gather call
CPC = CALL // P           # chunks per call (32)
OHG = 32                  # one-hot chunks per DMA group
F32 = mybir.dt.float32
BF16 = mybir.dt.bfloat16
FP8 = mybir.dt.float8e4
NP_BF16 = mybir.dt.np(BF16)
NP_FP8 = mybir.dt.np(FP8)
I16 = mybir.dt.int16


# ----------------------------------------------------------------------------
# host-side preprocessing
# ----------------------------------------------------------------------------

def _permute_nodes(dst):
    """Degree-balanced node permutation: sort by in-degree, deal round-robin
    over the 784 (core, block) windows.  Returns pos[n] in [0, NPAD)."""
    deg = np.bincount(dst, minlength=N)
    order = np.argsort(-deg, kind="stable")
    rank = np.empty(N, np.int64)
    rank[order] = np.arange(N)
    nwin = NCORES * NPB
    win = rank % nwin
    slot = rank // nwin
    core = win % NCORES
    block = win // NCORES
    pos = core * NPC + block * P + slot
    return pos, deg


def _build_streams(sp, dp):
    """Shared chunk structure + per-core packed idx streams and one-hots.

    sp, dp: permuted source / dest position per edge.
    Returns (cprog [NQ, NPB] shared chunks-per-cell, ncalls [NQ],
             per-core dict k -> (idx16 list per q, ohp array)).
    """
    q = np.searchsorted(np.array(QBOUND), sp, side="right").astype(np.int64)
    sidx = sp - np.array(QBASE)[q]                 # unsigned in [0, 32768)
    core = dp // NPC
    blk = (dp % NPC) // P
    slot = (dp % P).astype(np.int64)

    counts = np.zeros((NCORES, NQ, NPB), np.int64)
    per = {}
    for k in range(NCORES):
        mk = core == k
        for qq in range(NQ):
            m = mk & (q == qq)
            sb = blk[m]
            o = np.lexsort((sidx[m], sb))
            per[(k, qq)] = (sidx[m][o], sb[o], slot[m][o])
            counts[k, qq] = np.bincount(sb, minlength=NPB)

    cpb = -(-counts.max(axis=0) // P)              # [NQ, NPB]
    cpb[0] = np.maximum(cpb[0], 1)                 # guarantee >=1 chunk/block
    # stream lengths (padded to CALL), shared across cores
    slen = cpb.sum(axis=1) * P
    ncalls = -(-slen // CALL)
    ncalls = np.maximum(ncalls, 1)

    # chunk numbering in program order (b-major, q inner)
    chunks_per_block = cpb.sum(axis=0)             # [NPB]
    chunk_base = np.concatenate([[0], np.cumsum(chunks_per_block)[:-1]])
    CHT = int(chunks_per_block.sum())
    # cell offset of (q, b) within stream q (in edge positions)
    cell_off = np.concatenate(
        [np.zeros((NQ, 1), np.int64), np.cumsum(cpb * P, axis=1)[:, :-1]],
        axis=1)
    # chunk offset of stream q within block b (program order: q ascending)
    qoff = np.concatenate(
        [np.zeros((1, NPB), np.int64), np.cumsum(cpb, axis=0)[:-1]], axis=0)

    nohc = -(-CHT // OHG)
    percore = {}
    for k in range(NCORES):
        idx16s = []
        ohp = np.zeros((nohc, P, OHG, P), NP_FP8)
        for qq in range(NQ):
            si, sb, sl = per[(k, qq)]
            tot = int(ncalls[qq]) * CALL
            sarr = np.zeros(tot, np.int16)
            # scatter edges into their padded cell spans
            r = np.arange(len(sb)) - np.concatenate(
                [[0], np.cumsum(np.bincount(sb, minlength=NPB))[:-1]])[sb]
            epos = cell_off[qq][sb] + r
            sarr[epos] = si.astype(np.int16)
            # tail: -1 (trailing negatives are trimmed by ucode)
            used = int(cpb[qq].sum()) * P
            if used < tot:
                sarr[used:] = -1
            idx16s.append(np.concatenate([
                np.tile(sarr[c * CALL:(c + 1) * CALL]
                        .reshape(CALL // 16, 16).T, (8, 1))
                for c in range(int(ncalls[qq]))], axis=0))
            # one-hot entries
            ch = chunk_base[sb] + qoff[qq][sb] + r // P
            ohp[ch // OHG, r % P, ch % OHG, sl] = 1.0
        percore[k] = (idx16s, ohp)
    return cpb, ncalls, chunk_base, CHT, nohc, percore


# ----------------------------------------------------------------------------
# device programs
# ----------------------------------------------------------------------------

def _emit_streams(nc, tc, ctx, src_param, bases, ncalls, name):
    """Per-stream gather machinery: returns consume(q) -> (pay_tile, chunk)."""
    pay_pools = [
        ctx.enter_context(tc.tile_pool(name=f"{name}_pay{q}", bufs=2))
        for q in range(NQ)
    ]
    meta_pools = [
        ctx.enter_context(tc.tile_pool(name=f"{name}_meta{q}", bufs=2))
        for q in range(NQ)
    ]

    class Stream:
        def __init__(self, q):
            self.q = q
            self.next_chunk = 0
            self.cur_call = -1
            self.pay = None

        def ensure(self, idx_ins):
            call = self.next_chunk // CPC
            if call != self.cur_call:
                self.cur_call = call
                q = self.q
                idx_t = meta_pools[q].tile([P, CALL // 16], I16, tag="idx")
                nc.sync.dma_start(
                    out=idx_t[:], in_=idx_ins[q][call * P:(call + 1) * P, :])
                self.pay = pay_pools[q].tile([P, CPC, DIN], BF16, tag="pay")
                lo, hi = bases[q]
                nc.gpsimd.dma_gather(
                    self.pay[:], src_param[lo:hi, :], idx_t[:],
                    CALL, CALL, DIN, single_packet=False, queue_num=q)

        def consume(self, idx_ins):
            self.ensure(idx_ins)
            t = self.next_chunk
            self.next_chunk += 1
            return self.pay, t % CPC

    return [Stream(q) for q in range(NQ)]


def _emit_oh(nc, tc, ctx, oh_in, nohc, name):
    """One-hot group loader: consume() -> (oh_tile, col)."""
    pool = ctx.enter_context(tc.tile_pool(name=f"{name}_oh", bufs=2))

    class OH:
        def __init__(self):
            self.next_chunk = 0
            self.cur_grp = -1
            self.tile = None

        def consume(self):
            g = self.next_chunk // OHG
            if g != self.cur_grp:
                self.cur_grp = g
                self.tile = pool.tile([P, OHG, P], FP8, tag="oh")
                nc.sync.dma_start(out=self.tile[:], in_=oh_in[g, :, :, :])
            t = self.next_chunk
            self.next_chunk += 1
            return self.tile, t % OHG

    return OH()


def _declare_common(nc, cprog, ncalls, nohc, src_shape):
    src = nc.declare_dram_parameter("src", list(src_shape), BF16,
                                    isOutput=False)
    idx_ins = [
        nc.declare_dram_parameter(f"idx{q}", [int(ncalls[q]) * P, CALL // 16],
                                  I16, isOutput=False)
        for q in range(NQ)
    ]
    oh_in = nc.declare_dram_parameter("oh", [nohc, P, OHG, P], FP8,
                                      isOutput=False)
    dinv_in = nc.declare_dram_parameter("dinvt", [P, NPB], F32, isOutput=False)
    blk_in = nc.declare_dram_parameter("blk", [NPC, DIN], BF16, isOutput=False)
    return src, idx_ins, oh_in, dinv_in, blk_in


def _gather_bases(src_rows):
    """Stream q's dma_gather source slice [QBASE[q], QBOUND[q])."""
    return [(QBASE[q], min(QBOUND[q], src_rows)) for q in range(NQ)]


def build_launch1(cprog, ncalls, nohc):
    """GIN aggregation + MLP + head matmuls -> yhat rows."""
    nc = bacc.Bacc(dynamic_dma_scratch_size=65536, num_swdge_queues=4)
    src, idx_ins, oh_in, dinv_in, xblk_in = _declare_common(
        nc, cprog, ncalls, nohc, [NPAD, DIN])
    w1_in = nc.declare_dram_parameter("w1", [DIN, DH], BF16, isOutput=False)
    w2_in = nc.declare_dram_parameter("w2", [DH, DH], BF16, isOutput=False)
    w3_in = nc.declare_dram_parameter("w3", [DH, 2 * DOUT], BF16,
                                      isOutput=False)
    vec_in = nc.declare_dram_parameter("vecs", [DH, 3], F32, isOutput=False)
    ib_in = nc.declare_dram_parameter("ib16", [P, P], BF16, isOutput=False)
    y_out = nc.declare_dram_parameter("y", [NPC, 2 * DOUT], BF16,
                                      isOutput=True)

    with ExitStack() as ctx:
        tc = ctx.enter_context(tile.TileContext(nc))
        wp = ctx.enter_context(tc.tile_pool(name="weights", bufs=1))
        w1 = wp.tile([DIN, DH], BF16, tag="w1")
        nc.sync.dma_start(out=w1[:], in_=w1_in[:])
        w2 = wp.tile([DH, DH], BF16, tag="w2")
        nc.sync.dma_start(out=w2[:], in_=w2_in[:])
        w3 = wp.tile([DH, 2 * DOUT], BF16, tag="w3")
        nc.sync.dma_start(out=w3[:], in_=w3_in[:])
        vcols = wp.tile([DH, 3], F32, tag="vcols")
        nc.sync.dma_start(out=vcols[:], in_=vec_in[:])
        ib16 = wp.tile([P, P], BF16, tag="ib16")
        nc.sync.dma_start(out=ib16[:], in_=ib_in[:])
        dinvt = wp.tile([P, NPB], F32, tag="dinvt")
        nc.sync.dma_start(out=dinvt[:], in_=dinv_in[:])
        s_col = vcols[:, 0:1]
        t_col = vcols[:, 1:2]
        b2_col = vcols[:, 2:3]

        xbp = ctx.enter_context(tc.tile_pool(name="xblk", bufs=2))
        mlp = ctx.enter_context(tc.tile_pool(name="mlp", bufs=2))
        mpsum = ctx.enter_context(
            tc.tile_pool(name="mpsum", bufs=2, space="PSUM"))
        tpsum = ctx.enter_context(
            tc.tile_pool(name="tpsum", bufs=2, space="PSUM"))
        apsum = ctx.enter_context(
            tc.tile_pool(name="apsum", bufs=2, space="PSUM"))

        streams = _emit_streams(nc, tc, ctx, src, _gather_bases(NPAD),
                                ncalls, "l1")
        ohs = _emit_oh(nc, tc, ctx, oh_in, nohc, "l1")

        for b in range(NPB):
            nch = int(cprog[0][b] + cprog[1][b])
            psum = apsum.tile([DIN, P], F32, tag="agg")
            x_blk = xbp.tile([P, DIN], BF16, tag="xb")
            nc.sync.dma_start(out=x_blk[:], in_=xblk_in[b * P:(b + 1) * P, :])
            nc.tensor.matmul(psum[:], lhsT=x_blk[:], rhs=ib16[:],
                             start=True, stop=(nch == 0))
            done = 0
            for q in range(NQ):
                st = streams[q]
                for _ in range(int(cprog[q][b])):
                    pay, cl = st.consume(idx_ins)
                    oht, oc = ohs.consume()
                    nc.tensor.matmul(
                        psum[:], lhsT=pay[:, cl, :], rhs=oht[:, oc, :],
                        start=False, stop=(done == nch - 1))
                    done += 1
            # MLP: h = relu(s*(W1^T h0) + t); h = relu(W2^T h + b2)
            h0 = mlp.tile([DIN, P], BF16, tag="h0")
            nc.scalar.activation(h0[:], psum[:],
                                 mybir.ActivationFunctionType.Copy)
            p2 = mpsum.tile([DH, P], F32, tag="mp")
            nc.tensor.matmul(p2[:], lhsT=w1[:], rhs=h0[:], start=True,
                             stop=True)
            h1 = mlp.tile([DH, P], BF16, tag="h1")
            nc.scalar.activation(h1[:], p2[:],
                                 mybir.ActivationFunctionType.Relu,
                                 bias=t_col, scale=s_col)
            p3 = mpsum.tile([DH, P], F32, tag="mp")
            nc.tensor.matmul(p3[:], lhsT=w2[:], rhs=h1[:], start=True,
                             stop=True)
            h2 = mlp.tile([DH, P], BF16, tag="h2")
            nc.scalar.activation(h2[:], p3[:],
                                 mybir.ActivationFunctionType.Relu,
                                 bias=b2_col, scale=1.0)
            p4 = mpsum.tile([2 * DOUT, P], F32, tag="mp")
            nc.tensor.matmul(p4[:], lhsT=w3[:], rhs=h2[:], start=True,
                             stop=True)
            yt = mlp.tile([2 * DOUT, P], BF16, tag="yt")
            nc.scalar.activation(yt[:], p4[:],
                                 mybir.ActivationFunctionType.Copy)
            p5 = tpsum.tile([P, 2 * DOUT], BF16, tag="p5")
            nc.tensor.transpose(p5[:], yt[:], ib16[:])
            yn = mlp.tile([P, 2 * DOUT], BF16, tag="yn")
            nc.scalar.activation(yn[:], p5[:],
                                 mybir.ActivationFunctionType.Copy,
                                 scale=dinvt[:, b:b + 1])
            nc.sync.dma_start(out=y_out[b * P:(b + 1) * P, :], in_=yn[:])
    nc.finalize()
    return nc


def build_launch2(cprog, ncalls, nohc):
    """GCN aggregation of yhat rows (node-major) + self + bias."""
    nc = bacc.Bacc(dynamic_dma_scratch_size=65536, num_swdge_queues=4)
    src, idx_ins, oh_in, dinv_in, yblk_in = _declare_common(
        nc, cprog, ncalls, nohc, [NPAD, 2 * DOUT])
    bias_in = nc.declare_dram_parameter("biasb", [P, 2 * DOUT], F32,
                                        isOutput=False)
    out = nc.declare_dram_parameter("out", [NPC, 2 * DOUT], F32,
                                    isOutput=True)

    with ExitStack() as ctx:
        tc = ctx.enter_context(tile.TileContext(nc))
        wp = ctx.enter_context(tc.tile_pool(name="consts", bufs=1))
        bias_t = wp.tile([P, 2 * DOUT], F32, tag="bias")
        nc.sync.dma_start(out=bias_t[:], in_=bias_in[:])
        dinvt = wp.tile([P, NPB], F32, tag="dinvt")
        nc.sync.dma_start(out=dinvt[:], in_=dinv_in[:])

        ybp = ctx.enter_context(tc.tile_pool(name="yblk", bufs=2))
        fin = ctx.enter_context(tc.tile_pool(name="fin", bufs=2))
        apsum = ctx.enter_context(
            tc.tile_pool(name="apsum", bufs=2, space="PSUM"))

        streams = _emit_streams(nc, tc, ctx, src, _gather_bases(NPAD),
                                ncalls, "l2")
        ohs = _emit_oh(nc, tc, ctx, oh_in, nohc, "l2")

        for b in range(NPB):
            nch = int(cprog[0][b] + cprog[1][b])
            psum = apsum.tile([P, 2 * DOUT], F32, tag="agg")
            done = 0
            for q in range(NQ):
                st = streams[q]
                for _ in range(int(cprog[q][b])):
                    pay, cl = st.consume(idx_ins)
                    oht, oc = ohs.consume()
                    nc.tensor.matmul(
                        psum[:], lhsT=oht[:, oc, :], rhs=pay[:, cl, :],
                        start=(done == 0), stop=(done == nch - 1))
                    done += 1
            yb = ybp.tile([P, 2 * DOUT], BF16, tag="yb")
            nc.sync.dma_start(out=yb[:], in_=yblk_in[b * P:(b + 1) * P, :])
            t1 = fin.tile([P, 2 * DOUT], F32, tag="t1")
            nc.vector.scalar_tensor_tensor(
                out=t1[:], in0=yb[:], scalar=dinvt[:, b:b + 1], in1=bias_t[:],
                op0=mybir.AluOpType.mult, op1=mybir.AluOpType.add)
            ob = fin.tile([P, 2 * DOUT], F32, tag="ob")
            nc.vector.scalar_tensor_tensor(
                out=ob[:], in0=psum[:], scalar=dinvt[:, b:b + 1], in1=t1[:],
                op0=mybir.AluOpType.mult, op1=mybir.AluOpType.add)
            nc.sync.dma_start(out=out[b * P:(b + 1) * P, :], in_=ob[:])
    nc.finalize()
    return nc


# ----------------------------------------------------------------------------
# entry point
# ----------------------------------------------------------------------------

_CACHE = {}
LAST_TIMES = {}


def make_in_maps1(prep):
    in_maps = []
    for k in range(NCORES):
        idx16s, ohp = prep["percore"][k]
        m = {"src": prep["x_pos"], "oh": ohp,
             "blk": np.ascontiguousarray(prep["x_pos"][k * NPC:(k + 1) * NPC]),
             "w1": prep["W1"], "w2": prep["W2"], "w3": prep["w3"],
             "vecs": prep["vecs"], "ib16": prep["ib16"],
             "dinvt": prep["dinvt"][k]}
        for q in range(NQ):
            m[f"idx{q}"] = idx16s[q]
        in_maps.append(m)
    return in_maps


def make_in_maps2(prep, y_full):
    in_maps = []
    for k in range(NCORES):
        idx16s, ohp = prep["percore"][k]
        m = {"src": y_full, "oh": ohp,
             "blk": y_full[k * NPC:(k + 1) * NPC],
             "biasb": prep["biasb"], "dinvt": prep["dinvt"][k]}
        m["blk"] = np.ascontiguousarray(m["blk"])
        for q in range(NQ):
            m[f"idx{q}"] = idx16s[q]
        in_maps.append(m)
    return in_maps


def _prepare(x, edge_index, W1, b1, gamma, beta, rmean, rvar, W2, b2,
             Wmu, bmu, Wls, bls):
    src = np.ascontiguousarray(edge_index[0]).astype(np.int64)
    dst = np.ascontiguousarray(edge_index[1]).astype(np.int64)
    pos, deg_in = _permute_nodes(dst)

    sp = pos[src]
    dp = pos[dst]
    cpb, ncalls, chunk_base, CHT, nohc, percore = _build_streams(sp, dp)

    # dense data, permuted layout
    x_pos = np.zeros((NPAD, DIN), NP_BF16)
    x_pos[pos[:N]] = x.astype(NP_BF16)

    deg = deg_in.astype(np.float64) + 1.0
    dinv = (1.0 / np.sqrt(deg)).astype(np.float32)
    dinv_pos = np.zeros(NPAD, np.float32)
    dinv_pos[pos[:N]] = dinv
    # dinvt[k][p, b] = dinv of (core k, block b, slot p)
    dinvt = [
        np.ascontiguousarray(
            dinv_pos[k * NPC:(k + 1) * NPC].reshape(NPB, P).T)
        for k in range(NCORES)
    ]

    eps = 1e-5
    s64 = gamma.astype(np.float64) / np.sqrt(rvar.astype(np.float64) + eps)
    t64 = s64 * (b1.astype(np.float64) - rmean.astype(np.float64)) \
        + beta.astype(np.float64)
    vecs = np.ascontiguousarray(
        np.stack([s64.astype(np.float32), t64.astype(np.float32),
                  b2.astype(np.float32)], axis=1))
    w3 = np.concatenate([Wmu, Wls], axis=1).astype(NP_BF16)
    biasb = np.tile(np.concatenate([bmu, bls]).astype(np.float32)[None, :],
                    (P, 1))
    return dict(pos=pos, cpb=cpb, ncalls=ncalls, nohc=nohc, percore=percore,
                x_pos=x_pos, dinvt=dinvt, vecs=vecs, biasb=biasb,
                W1=np.ascontiguousarray(W1.astype(NP_BF16)),
                W2=np.ascontiguousarray(W2.astype(NP_BF16)),
                w3=np.ascontiguousarray(w3),
                ib16=np.eye(P, dtype=NP_BF16),
                l1=(None, cpb, ncalls, nohc), l2=(None, cpb, ncalls, nohc))


def kernel(**inputs):
    key = hashlib.sha1(
        np.ascontiguousarray(inputs["edge_index"]).tobytes()).hexdigest()
    if key not in _CACHE:
        prep = _prepare(**inputs)
        nc1 = build_launch1(prep["cpb"], prep["ncalls"], prep["nohc"])
        nc2 = build_launch2(prep["cpb"], prep["ncalls"], prep["nohc"])
        _CACHE[key] = (prep, nc1, nc2)
    prep, nc1, nc2 = _CACHE[key]

    in_maps1 = make_in_maps1(prep)
    t0 = time.time()
    res1 = run_bass_kernel_spmd(nc1, in_maps1, list(range(NCORES)))
    LAST_TIMES["launch1_wall_s"] = time.time() - t0
    y_full = np.concatenate([res1.results[k]["y"] for k in range(NCORES)],
                            axis=0)

    in_maps2 = make_in_maps2(prep, y_full)
    t0 = time.time()
    res2 = run_bass_kernel_spmd(nc2, in_maps2, list(range(NCORES)))
    LAST_TIMES["launch2_wall_s"] = time.time() - t0
    out_full = np.concatenate([res2.results[k]["out"] for k in range(NCORES)],
                              axis=0)

    final = out_full[prep["pos"][:N]]
    return np.ascontiguousarray(final[:, :DOUT]), \
        np.ascontiguousarray(final[:, DOUT:])
